# revision 1
# baseline (speedup 1.0000x reference)
"""Trainium2 Bass kernel for nn_PositionalEncoding_61151744360729.

out[b, s, n, :] = x[b, s, n, :] + ||x[b, s+1, n, :] - x[b, s, n, :]||_2
(with distance 0 at s = S-1).

Sharding: data-parallel on batch across 8 NeuronCores (64 batches/core).
On-core layout: partition p = b*2 + h (b = batch, h = sequence half),
free dim = frames*75 floats, so every DMA is a large contiguous span per
partition and the outermost AP dim (64) lets SWDGE fan descriptors over
all 16 SDMA engines. Each batch is padded host-side with a copy of its
last frame, which makes the last-frame distance exactly 0 with no
special-casing. Per 64-frame chunk: DVE shifted subtract -> ACT square
-> two strided DVE adds (sum over the 3 coords) -> ACT sqrt -> three
strided DVE broadcast-adds -> DMA out.
"""

import sys
from contextlib import ExitStack

for _p in ("/opt/trn_rl_repo", "/root/.axon_site/_ro/trn_rl_repo"):
    if _p not in sys.path:
        sys.path.insert(0, _p)

import numpy as np

import concourse.bass as bass
import concourse.tile as tile
from concourse import bacc, mybir
from concourse.bass_utils import run_bass_kernel_spmd

B, S, N, C = 512, 1024, 25, 3
FW = N * C                 # 75 floats per frame
NCORES = 8
BC = B // NCORES           # 64 batches per core
H = 2                      # sequence halves -> 128 partitions
SH = S // H                # 512 frames per half
P = H * BC                 # 128 partitions
F = 64                     # frames per chunk per partition
K = SH // F                # 8 chunks
IN_FLAT = BC * (S + 1) * FW   # input padded by one zero frame per batch
OUT_FLAT = BC * S * FW

_cache = {}


def _build():
    f32 = mybir.dt.float32
    Af = mybir.ActivationFunctionType
    nc = bacc.Bacc(
        "TRN2", target_bir_lowering=False, debug=False, num_devices=NCORES
    )
    xin = nc.dram_tensor("xin", [IN_FLAT], f32, kind="ExternalInput")
    yout = nc.dram_tensor("yout", [OUT_FLAT], f32, kind="ExternalOutput")

    with tile.TileContext(nc) as tc, ExitStack() as ctx:
        pin = ctx.enter_context(tc.tile_pool(name="pin", bufs=4))
        pmid = ctx.enter_context(tc.tile_pool(name="pmid", bufs=2))
        psm = ctx.enter_context(tc.tile_pool(name="psm", bufs=3))
        pout = ctx.enter_context(tc.tile_pool(name="pout", bufs=2))

        PF = 3  # input prefetch depth

        def issue_in(k):
            t = pin.tile([P, (F + 1) * FW], f32)
            src = bass.AP(
                xin,
                k * F * FW,
                [[(S + 1) * FW, BC], [SH * FW, H], [1, (F + 1) * FW]],
            )
            nc.gpsimd.dma_start(t[:], src)
            return t

        in_tiles = [issue_in(k) for k in range(PF)]

        for k in range(K):
            in_t = in_tiles[k]

            diff_t = pmid.tile([P, F * FW], f32)
            nc.vector.tensor_sub(
                diff_t[:], in_t[:, FW:(F + 1) * FW], in_t[:, 0:F * FW]
            )
            nc.scalar.activation(diff_t[:], diff_t[:], Af.Square)

            sq4 = diff_t[:].rearrange("p (f n c) -> p f n c", f=F, n=N, c=C)
            dist2_t = psm.tile([P, F * N], f32)
            d2 = dist2_t[:].rearrange("p (f n) -> p f n", f=F)
            nc.vector.tensor_add(d2, sq4[:, :, :, 0], sq4[:, :, :, 1])
            nc.vector.tensor_add(d2, d2, sq4[:, :, :, 2])
            # sqrt in place: dist2_t becomes dist
            nc.scalar.activation(dist2_t[:], dist2_t[:], Af.Sqrt)
            dist_t = dist2_t

            if k + PF < K:
                in_tiles.append(issue_in(k + PF))

            out_t = pout.tile([P, F * FW], f32)
            out4 = out_t[:].rearrange("p (f n c) -> p f n c", f=F, n=N, c=C)
            in4 = in_t[:, 0:F * FW].rearrange(
                "p (f n c) -> p f n c", f=F, n=N, c=C
            )
            dvb = (
                dist_t[:]
                .rearrange("p (f n) -> p f n", f=F)
                .unsqueeze(3)
                .broadcast_to([P, F, N, C])
            )
            nc.vector.tensor_add(out4, in4, dvb)

            dst = bass.AP(
                yout,
                k * F * FW,
                [[S * FW, BC], [SH * FW, H], [1, F * FW]],
            )
            nc.gpsimd.dma_start(dst, out_t[:])

    nc.compile()
    return nc


def kernel(x: np.ndarray, **_unused) -> np.ndarray:
    x = np.ascontiguousarray(np.asarray(x), dtype=np.float32)
    assert x.shape == (B, S, N, C), x.shape

    if "nc" not in _cache:
        _cache["nc"] = _build()
    nc = _cache["nc"]

    in_maps = []
    for ci in range(NCORES):
        xc = x[ci * BC:(ci + 1) * BC].reshape(BC, S * FW)
        xp = np.empty((BC, (S + 1) * FW), dtype=np.float32)
        xp[:, : S * FW] = xc
        # pad frame = copy of the last frame -> diff at s = S-1 is 0
        xp[:, S * FW:] = xc[:, (S - 1) * FW:]
        in_maps.append({"xin": xp.reshape(IN_FLAT)})

    res = run_bass_kernel_spmd(nc, in_maps, core_ids=list(range(NCORES)))
    _cache["last_results"] = res

    out = np.empty((B, S, N, C), dtype=np.float32)
    for ci in range(NCORES):
        out[ci * BC:(ci + 1) * BC] = res.results[ci]["yout"].reshape(
            BC, S, N, C
        )
    return out



# revision 2
# speedup vs baseline: 1.5760x; 1.5760x over previous
"""Trainium2 Bass kernel for nn_PositionalEncoding_61151744360729.

out[b, s, n, :] = x[b, s, n, :] + ||x[b, s+1, n, :] - x[b, s, n, :]||_2
(with distance 0 at s = S-1).

Sharding: data-parallel on batch across 8 NeuronCores (64 batches/core).

Device layout (v2): fp16 end-to-end, c-planar. Host repacks x to, per
(batch, seq-half) partition, [3 coord planes][SH+1 frames][26 nodes]
(nodes padded 25->26, one extra frame = copy of the last so the final
distance is exactly 0). With this layout every DVE tensor_tensor op is
unit-stride, 4B-aligned fp16 -> 2x perf mode:
  sub (shift by one 52B frame) -> ACT Square -> two plane adds
  -> ACT Sqrt -> three plane broadcast-adds -> DMA out.
fp16 I/O also halves HBM traffic vs the fp32 baseline.
"""

import sys
from contextlib import ExitStack

for _p in ("/opt/trn_rl_repo", "/root/.axon_site/_ro/trn_rl_repo"):
    if _p not in sys.path:
        sys.path.insert(0, _p)

import numpy as np

import concourse.bass as bass
import concourse.tile as tile
from concourse import bacc, mybir
from concourse.bass_utils import run_bass_kernel_spmd

B, S, N, C = 512, 1024, 25, 3
NCORES = 8
BC = B // NCORES           # 64 batches per core
H = 2                      # sequence halves -> 128 partitions
SH = S // H                # 512 frames per half
P = H * BC                 # 128 partitions
NP = 26                    # nodes padded to 26 (4B-aligned frame stride)
F = 128                    # frames per chunk per partition
K = SH // F                # 4 chunks
IN_PLANE = (SH + 1) * NP   # 13338 elems per coord plane per partition
OUT_PLANE = SH * NP        # 13312
IN_FLAT = P * C * IN_PLANE
OUT_FLAT = P * C * OUT_PLANE

_cache = {}


def _build():
    f16 = mybir.dt.float16
    Af = mybir.ActivationFunctionType
    nc = bacc.Bacc(
        "TRN2", target_bir_lowering=False, debug=False, num_devices=NCORES
    )
    xin = nc.dram_tensor("xin", [IN_FLAT], f16, kind="ExternalInput")
    yout = nc.dram_tensor("yout", [OUT_FLAT], f16, kind="ExternalOutput")

    FI = (F + 1) * NP      # 3354 input elems per plane per chunk
    FO = F * NP            # 3328 output elems per plane per chunk

    with tile.TileContext(nc) as tc, ExitStack() as ctx:
        pin = ctx.enter_context(tc.tile_pool(name="pin", bufs=3))
        pmid = ctx.enter_context(tc.tile_pool(name="pmid", bufs=2))
        psm = ctx.enter_context(tc.tile_pool(name="psm", bufs=2))
        pout = ctx.enter_context(tc.tile_pool(name="pout", bufs=2))

        PF = 2  # input prefetch depth

        def issue_in(k):
            t = pin.tile([P, C * FI], f16)
            src = bass.AP(
                xin,
                k * F * NP,
                [[C * IN_PLANE, P], [IN_PLANE, C], [1, FI]],
            )
            nc.gpsimd.dma_start(t[:], src)
            return t

        in_tiles = [issue_in(k) for k in range(PF)]

        for k in range(K):
            in_t = in_tiles[k]
            in3 = in_t[:].rearrange("p (c x) -> p c x", c=C)

            # diff: shift by one frame (26 elems = 52B, 4B-aligned) -> 2x
            d_t = pmid.tile([P, C * FO], f16)
            d3 = d_t[:].rearrange("p (c x) -> p c x", c=C)
            nc.vector.tensor_sub(d3, in3[:, :, NP:], in3[:, :, 0:FO])

            # square in place on the scalar engine
            nc.scalar.activation(d_t[:], d_t[:], Af.Square)

            # dist2 = sum over the 3 coord planes (unit-stride adds -> 2x)
            s_t = psm.tile([P, FO], f16)
            nc.vector.tensor_add(s_t[:], d3[:, 0], d3[:, 1])
            nc.vector.tensor_add(s_t[:], s_t[:], d3[:, 2])

            # dist = sqrt(dist2) in place
            nc.scalar.activation(s_t[:], s_t[:], Af.Sqrt)

            if k + PF < K:
                in_tiles.append(issue_in(k + PF))

            # out_c = x_c + dist, one unit-stride 2x add per coord plane
            o_t = pout.tile([P, C * FO], f16)
            o3 = o_t[:].rearrange("p (c x) -> p c x", c=C)
            for c in range(C):
                nc.vector.tensor_add(o3[:, c], in3[:, c, 0:FO], s_t[:])

            dst = bass.AP(
                yout,
                k * F * NP,
                [[C * OUT_PLANE, P], [OUT_PLANE, C], [1, FO]],
            )
            nc.gpsimd.dma_start(dst, o_t[:])

    nc.compile()
    return nc


def kernel(x: np.ndarray, **_unused) -> np.ndarray:
    x = np.asarray(x)
    assert x.shape == (B, S, N, C), x.shape

    if "nc" not in _cache:
        _cache["nc"] = _build()
    nc = _cache["nc"]

    # Host-side repack: fp16, per (batch, half) partition a c-planar
    # [3, SH+1, 26] block; frame SH is the next real frame (half 0) or a
    # copy of the last frame (half 1) so the device-side distance at the
    # true sequence end is exactly 0.
    xh = np.ascontiguousarray(x).astype(np.float16)          # [B,S,25,3]
    ext = np.concatenate([xh, xh[:, -1:]], axis=1)           # [B,S+1,25,3]
    h0 = ext[:, 0:SH + 1]                                    # [B,513,25,3]
    h1 = ext[:, SH:S + 1]                                    # [B,513,25,3]
    hv = np.stack([h0, h1], axis=1)                          # [B,2,513,25,3]
    pl = np.transpose(hv, (0, 1, 4, 2, 3))                   # [B,2,3,513,25]
    buf = np.zeros((B, H, C, SH + 1, NP), np.float16)
    buf[..., :N] = pl

    in_maps = [
        {"xin": buf[ci * BC:(ci + 1) * BC].reshape(IN_FLAT)}
        for ci in range(NCORES)
    ]

    res = run_bass_kernel_spmd(nc, in_maps, core_ids=list(range(NCORES)))
    _cache["last_results"] = res

    out = np.empty((B, S, N, C), dtype=np.float32)
    for ci in range(NCORES):
        y = np.asarray(res.results[ci]["yout"]).reshape(BC, H, C, SH, NP)
        y = y[..., :N]                                       # strip node pad
        y = np.transpose(y, (0, 1, 3, 4, 2))                 # [BC,2,SH,25,3]
        out[ci * BC:(ci + 1) * BC] = y.reshape(BC, S, N, C).astype(np.float32)
    return out


# revision 4
# speedup vs baseline: 2.2838x; 1.4491x over previous
"""Trainium2 Bass kernel for nn_PositionalEncoding_61151744360729.

out[b, s, n, :] = x[b, s, n, :] + ||x[b, s+1, n, :] - x[b, s, n, :]||_2
(with distance 0 at s = S-1).

Sharding: data-parallel on batch across 8 NeuronCores (64 batches/core).

Device layout (v2): fp16 end-to-end, c-planar. Host repacks x to, per
(batch, seq-half) partition, [3 coord planes][SH+1 frames][26 nodes]
(nodes padded 25->26, one extra frame = copy of the last so the final
distance is exactly 0). With this layout every DVE tensor_tensor op is
unit-stride, 4B-aligned fp16 -> 2x perf mode:
  sub (shift by one 52B frame) -> ACT Square -> two plane adds
  -> ACT Sqrt -> three plane broadcast-adds -> DMA out.
fp16 I/O also halves HBM traffic vs the fp32 baseline.
"""

import sys
from contextlib import ExitStack

for _p in ("/opt/trn_rl_repo", "/root/.axon_site/_ro/trn_rl_repo"):
    if _p not in sys.path:
        sys.path.insert(0, _p)

import numpy as np

import concourse.bass as bass
import concourse.tile as tile
from concourse import bacc, mybir
from concourse.bass_utils import run_bass_kernel_spmd

B, S, N, C = 512, 1024, 25, 3
NCORES = 8
BC = B // NCORES           # 64 batches per core
H = 2                      # sequence halves -> 128 partitions
SH = S // H                # 512 frames per half
P = H * BC                 # 128 partitions
NP = 26                    # nodes padded to 26 (4B-aligned frame stride)
F = 64                     # frames per chunk per partition
K = SH // F                # 8 chunks
IN_PLANE = (SH + 1) * NP   # 13338 elems per coord plane per partition
OUT_PLANE = SH * NP        # 13312
IN_FLAT = P * C * IN_PLANE
OUT_FLAT = P * C * OUT_PLANE

_cache = {}


def _build():
    f16 = mybir.dt.float16
    Af = mybir.ActivationFunctionType
    nc = bacc.Bacc(
        "TRN2", target_bir_lowering=False, debug=False, num_devices=NCORES
    )
    xin = nc.dram_tensor("xin", [IN_FLAT], f16, kind="ExternalInput")
    yout = nc.dram_tensor("yout", [OUT_FLAT], f16, kind="ExternalOutput")

    FI = (F + 1) * NP      # 3354 input elems per plane per chunk
    FO = F * NP            # 3328 output elems per plane per chunk

    with tile.TileContext(nc) as tc, ExitStack() as ctx:
        pin = ctx.enter_context(tc.tile_pool(name="pin", bufs=5))
        pmid = ctx.enter_context(tc.tile_pool(name="pmid", bufs=3))
        psm = ctx.enter_context(tc.tile_pool(name="psm", bufs=3))
        pout = ctx.enter_context(tc.tile_pool(name="pout", bufs=3))

        PF = 4  # input prefetch depth

        def issue_in(k):
            t = pin.tile([P, C * FI], f16)
            src = bass.AP(
                xin,
                k * F * NP,
                [[C * IN_PLANE, P], [IN_PLANE, C], [1, FI]],
            )
            nc.gpsimd.dma_start(t[:], src)
            return t

        in_tiles = [issue_in(k) for k in range(PF)]

        for k in range(K):
            in_t = in_tiles[k]
            in3 = in_t[:].rearrange("p (c x) -> p c x", c=C)

            # diff: shift by one frame (26 elems = 52B, 4B-aligned) -> 2x
            d_t = pmid.tile([P, C * FO], f16)
            d3 = d_t[:].rearrange("p (c x) -> p c x", c=C)
            nc.vector.tensor_sub(d3, in3[:, :, NP:], in3[:, :, 0:FO])

            # square in place on the scalar engine
            nc.scalar.activation(d_t[:], d_t[:], Af.Square)

            # dist2 = sum over the 3 coord planes (unit-stride adds -> 2x)
            s_t = psm.tile([P, FO], f16)
            nc.vector.tensor_add(s_t[:], d3[:, 0], d3[:, 1])
            nc.vector.tensor_add(s_t[:], s_t[:], d3[:, 2])

            # dist = sqrt(dist2) in place
            nc.scalar.activation(s_t[:], s_t[:], Af.Sqrt)

            if k + PF < K:
                in_tiles.append(issue_in(k + PF))

            # out_c = x_c + dist, one unit-stride 2x add per coord plane
            o_t = pout.tile([P, C * FO], f16)
            o3 = o_t[:].rearrange("p (c x) -> p c x", c=C)
            for c in range(C):
                nc.vector.tensor_add(o3[:, c], in3[:, c, 0:FO], s_t[:])

            dst = bass.AP(
                yout,
                k * F * NP,
                [[C * OUT_PLANE, P], [OUT_PLANE, C], [1, FO]],
            )
            nc.gpsimd.dma_start(dst, o_t[:])

    nc.compile()
    return nc


def kernel(x: np.ndarray, **_unused) -> np.ndarray:
    x = np.asarray(x)
    assert x.shape == (B, S, N, C), x.shape

    if "nc" not in _cache:
        _cache["nc"] = _build()
    nc = _cache["nc"]

    # Host-side repack: fp16, per (batch, half) partition a c-planar
    # [3, SH+1, 26] block; frame SH is the next real frame (half 0) or a
    # copy of the last frame (half 1) so the device-side distance at the
    # true sequence end is exactly 0.
    xh = np.ascontiguousarray(x).astype(np.float16)          # [B,S,25,3]
    ext = np.concatenate([xh, xh[:, -1:]], axis=1)           # [B,S+1,25,3]
    h0 = ext[:, 0:SH + 1]                                    # [B,513,25,3]
    h1 = ext[:, SH:S + 1]                                    # [B,513,25,3]
    hv = np.stack([h0, h1], axis=1)                          # [B,2,513,25,3]
    pl = np.transpose(hv, (0, 1, 4, 2, 3))                   # [B,2,3,513,25]
    buf = np.zeros((B, H, C, SH + 1, NP), np.float16)
    buf[..., :N] = pl

    in_maps = [
        {"xin": buf[ci * BC:(ci + 1) * BC].reshape(IN_FLAT)}
        for ci in range(NCORES)
    ]

    res = run_bass_kernel_spmd(nc, in_maps, core_ids=list(range(NCORES)))
    _cache["last_results"] = res

    out = np.empty((B, S, N, C), dtype=np.float32)
    for ci in range(NCORES):
        y = np.asarray(res.results[ci]["yout"]).reshape(BC, H, C, SH, NP)
        y = y[..., :N]                                       # strip node pad
        y = np.transpose(y, (0, 1, 3, 4, 2))                 # [BC,2,SH,25,3]
        out[ci * BC:(ci + 1) * BC] = y.reshape(BC, S, N, C).astype(np.float32)
    return out


# revision 5
# speedup vs baseline: 2.3169x; 1.0145x over previous
"""Trainium2 Bass kernel for nn_PositionalEncoding_61151744360729.

out[b, s, n, :] = x[b, s, n, :] + ||x[b, s+1, n, :] - x[b, s, n, :]||_2
(with distance 0 at s = S-1).

Sharding: data-parallel on batch across 8 NeuronCores (64 batches/core).

Device layout: fp16 end-to-end, c-planar. Host repacks x to, per
(batch, seq-half) partition, [3 coord planes][SH+1 frames][26 nodes]
(nodes padded 25->26, one extra frame = copy of the last so the final
distance is exactly 0). With this layout every DVE tensor_tensor op is
unit-stride, 4B-aligned fp16 -> 2x perf mode, and fp16 I/O halves HBM
traffic vs fp32.

Engine split per chunk: DVE does the 3 per-plane frame-shift subtracts
and the 3 per-plane broadcast-adds; ACT squares each plane in place and
takes the final sqrt; the 3-plane sum runs on the otherwise-idle PE as
identity matmuls accumulating into PSUM (the sqrt doubles as the
PSUM->SBUF drain); outputs DMA out per plane.
"""

import sys
from contextlib import ExitStack

for _p in ("/opt/trn_rl_repo", "/root/.axon_site/_ro/trn_rl_repo"):
    if _p not in sys.path:
        sys.path.insert(0, _p)

import numpy as np

import concourse.bass as bass
import concourse.tile as tile
from concourse import bacc, mybir
from concourse.bass_utils import run_bass_kernel_spmd

B, S, N, C = 512, 1024, 25, 3
NCORES = 8
BC = B // NCORES           # 64 batches per core
H = 2                      # sequence halves -> 128 partitions
SH = S // H                # 512 frames per half
P = H * BC                 # 128 partitions
NP = 26                    # nodes padded to 26 (4B-aligned frame stride)
F = 64                     # frames per chunk per partition
K = SH // F                # 8 chunks
IN_PLANE = (SH + 1) * NP   # 13338 elems per coord plane per partition
OUT_PLANE = SH * NP        # 13312
IN_FLAT = P * C * IN_PLANE
OUT_FLAT = P * C * OUT_PLANE
PSUM_W = 512               # one PSUM bank of fp32 per matmul window

_cache = {}


def _build():
    f16 = mybir.dt.float16
    f32 = mybir.dt.float32
    Af = mybir.ActivationFunctionType
    nc = bacc.Bacc(
        "TRN2", target_bir_lowering=False, debug=False, num_devices=NCORES
    )
    xin = nc.dram_tensor("xin", [IN_FLAT], f16, kind="ExternalInput")
    ident = nc.dram_tensor("ident", [P * P], f16, kind="ExternalInput")
    yout = nc.dram_tensor("yout", [OUT_FLAT], f16, kind="ExternalOutput")

    FI = (F + 1) * NP      # input elems per plane per chunk
    FO = F * NP            # output elems per plane per chunk

    with tile.TileContext(nc) as tc, ExitStack() as ctx:
        pconst = ctx.enter_context(tc.tile_pool(name="pconst", bufs=1))
        pin = ctx.enter_context(tc.tile_pool(name="pin", bufs=5))
        pmid = ctx.enter_context(tc.tile_pool(name="pmid", bufs=3))
        psm = ctx.enter_context(tc.tile_pool(name="psm", bufs=3))
        pout = ctx.enter_context(tc.tile_pool(name="pout", bufs=3))
        ppsum = ctx.enter_context(
            tc.tile_pool(name="ppsum", bufs=2, space="PSUM")
        )

        id_t = pconst.tile([P, P], f16)
        nc.gpsimd.dma_start(id_t[:], bass.AP(ident, 0, [[P, P], [1, P]]))

        PF = 4  # input prefetch depth

        def issue_in(k):
            t = pin.tile([P, C * FI], f16)
            src = bass.AP(
                xin,
                k * F * NP,
                [[C * IN_PLANE, P], [IN_PLANE, C], [1, FI]],
            )
            nc.gpsimd.dma_start(t[:], src)
            return t

        in_tiles = [issue_in(k) for k in range(PF)]

        for k in range(K):
            in_t = in_tiles[k]
            in3 = in_t[:].rearrange("p (c x) -> p c x", c=C)

            # diff per plane: shift by one frame (26 elems = 52B) -> 2x
            d_t = pmid.tile([P, C * FO], f16)
            d3 = d_t[:].rearrange("p (c x) -> p c x", c=C)
            for c in range(C):
                nc.vector.tensor_sub(
                    d3[:, c], in3[:, c, NP:], in3[:, c, 0:FO]
                )
                # square in place on the scalar engine
                nc.scalar.activation(d3[:, c], d3[:, c], Af.Square)

            # dist2 = sum over the 3 coord planes: identity matmuls
            # accumulating into PSUM on the otherwise-idle tensor engine
            ps_t = ppsum.tile([P, FO], f32)
            for w0 in range(0, FO, PSUM_W):
                w1 = min(w0 + PSUM_W, FO)
                for c in range(C):
                    nc.tensor.matmul(
                        ps_t[:, w0:w1],
                        id_t[:],
                        d3[:, c, w0:w1],
                        start=(c == 0),
                        stop=(c == C - 1),
                    )

            # dist = sqrt(dist2), draining PSUM (fp32) -> SBUF fp16
            s_t = psm.tile([P, FO], f16)
            nc.scalar.activation(s_t[:], ps_t[:], Af.Sqrt)

            if k + PF < K:
                in_tiles.append(issue_in(k + PF))

            # out_c = x_c + dist; DMA each plane out as soon as it's ready
            o_t = pout.tile([P, C * FO], f16)
            o3 = o_t[:].rearrange("p (c x) -> p c x", c=C)
            for c in range(C):
                nc.vector.tensor_add(o3[:, c], in3[:, c, 0:FO], s_t[:])
                dst = bass.AP(
                    yout,
                    c * OUT_PLANE + k * F * NP,
                    [[C * OUT_PLANE, P], [1, FO]],
                )
                nc.gpsimd.dma_start(dst, o3[:, c])

    nc.compile()
    return nc


def kernel(x: np.ndarray, **_unused) -> np.ndarray:
    x = np.asarray(x)
    assert x.shape == (B, S, N, C), x.shape

    if "nc" not in _cache:
        _cache["nc"] = _build()
    nc = _cache["nc"]

    # Host-side repack: fp16, per (batch, half) partition a c-planar
    # [3, SH+1, 26] block; frame SH is the next real frame (half 0) or a
    # copy of the last frame (half 1) so the device-side distance at the
    # true sequence end is exactly 0.
    xh = np.ascontiguousarray(x).astype(np.float16)          # [B,S,25,3]
    ext = np.concatenate([xh, xh[:, -1:]], axis=1)           # [B,S+1,25,3]
    h0 = ext[:, 0:SH + 1]                                    # [B,513,25,3]
    h1 = ext[:, SH:S + 1]                                    # [B,513,25,3]
    hv = np.stack([h0, h1], axis=1)                          # [B,2,513,25,3]
    pl = np.transpose(hv, (0, 1, 4, 2, 3))                   # [B,2,3,513,25]
    buf = np.zeros((B, H, C, SH + 1, NP), np.float16)
    buf[..., :N] = pl

    eye = np.eye(P, dtype=np.float16).reshape(P * P)
    in_maps = [
        {
            "xin": buf[ci * BC:(ci + 1) * BC].reshape(IN_FLAT),
            "ident": eye,
        }
        for ci in range(NCORES)
    ]

    res = run_bass_kernel_spmd(nc, in_maps, core_ids=list(range(NCORES)))
    _cache["last_results"] = res

    out = np.empty((B, S, N, C), dtype=np.float32)
    for ci in range(NCORES):
        y = np.asarray(res.results[ci]["yout"]).reshape(BC, H, C, SH, NP)
        y = y[..., :N]                                       # strip node pad
        y = np.transpose(y, (0, 1, 3, 4, 2))                 # [BC,2,SH,25,3]
        out[ci * BC:(ci + 1) * BC] = y.reshape(BC, S, N, C).astype(np.float32)
    return out


# revision 6
# speedup vs baseline: 2.3930x; 1.0329x over previous
"""Trainium2 Bass kernel for nn_PositionalEncoding_61151744360729.

out[b, s, n, :] = x[b, s, n, :] + ||x[b, s+1, n, :] - x[b, s, n, :]||_2
(with distance 0 at s = S-1).

Sharding: data-parallel on batch across 8 NeuronCores (64 batches/core).

Device layout: fp16 end-to-end, c-planar. Host repacks x to, per
(batch, seq-half) partition, [3 coord planes][SH+1 frames][26 nodes]
(nodes padded 25->26, one extra frame = copy of the last so the final
distance is exactly 0). With this layout every DVE tensor_tensor op is
unit-stride, 4B-aligned fp16 -> 2x perf mode, and fp16 I/O halves HBM
traffic vs fp32.

Engine split per chunk: DVE does the 3 per-plane frame-shift subtracts
and the 3 per-plane broadcast-adds; ACT squares each plane in place and
takes the final sqrt; the 3-plane sum runs on the otherwise-idle PE as
identity matmuls accumulating into PSUM (the sqrt doubles as the
PSUM->SBUF drain); outputs DMA out per plane.
"""

import sys
from contextlib import ExitStack

for _p in ("/opt/trn_rl_repo", "/root/.axon_site/_ro/trn_rl_repo"):
    if _p not in sys.path:
        sys.path.insert(0, _p)

import numpy as np

import concourse.bass as bass
import concourse.tile as tile
from concourse import bacc, mybir
from concourse.bass_utils import run_bass_kernel_spmd

B, S, N, C = 512, 1024, 25, 3
NCORES = 8
BC = B // NCORES           # 64 batches per core
H = 2                      # sequence halves -> 128 partitions
SH = S // H                # 512 frames per half
P = H * BC                 # 128 partitions
NP = 26                    # nodes padded to 26 (4B-aligned frame stride)
F = 64                     # frames per chunk per partition
K = SH // F                # 8 chunks
IN_PLANE = (SH + 1) * NP   # 13338 elems per coord plane per partition
OUT_PLANE = SH * NP        # 13312
IN_FLAT = P * C * IN_PLANE
OUT_FLAT = P * C * OUT_PLANE
PSUM_W = 512               # one PSUM bank of fp32 per matmul window

_cache = {}


def _build():
    f16 = mybir.dt.float16
    f32 = mybir.dt.float32
    Af = mybir.ActivationFunctionType
    nc = bacc.Bacc(
        "TRN2", target_bir_lowering=False, debug=False, num_devices=NCORES
    )
    xin = nc.dram_tensor("xin", [IN_FLAT], f16, kind="ExternalInput")
    ident = nc.dram_tensor("ident", [P * P], f16, kind="ExternalInput")
    yout = nc.dram_tensor("yout", [OUT_FLAT], f16, kind="ExternalOutput")

    FI = (F + 1) * NP      # input elems per plane per chunk
    FO = F * NP            # output elems per plane per chunk

    with tile.TileContext(nc) as tc, ExitStack() as ctx:
        pconst = ctx.enter_context(tc.tile_pool(name="pconst", bufs=1))
        pin = ctx.enter_context(tc.tile_pool(name="pin", bufs=7))
        pmid = ctx.enter_context(tc.tile_pool(name="pmid", bufs=3))
        psm = ctx.enter_context(tc.tile_pool(name="psm", bufs=4))
        pout = ctx.enter_context(tc.tile_pool(name="pout", bufs=4))
        ppsum = ctx.enter_context(
            tc.tile_pool(name="ppsum", bufs=2, space="PSUM")
        )

        PF = 5  # input prefetch depth

        def issue_in(k):
            # per-plane DMAs on the idle SP engine's HWDGE ring: finer
            # dependency granularity and no Q7 descriptor-gen latency
            t = pin.tile([P, C * FI], f16)
            t3 = t[:].rearrange("p (c x) -> p c x", c=C)
            for c in range(C):
                src = bass.AP(
                    xin,
                    c * IN_PLANE + k * F * NP,
                    [[C * IN_PLANE, P], [1, FI]],
                )
                nc.sync.dma_start(t3[:, c], src)
            return t

        in_tiles = [issue_in(0)]
        id_t = pconst.tile([P, P], f16)
        nc.sync.dma_start(id_t[:], bass.AP(ident, 0, [[P, P], [1, P]]))
        in_tiles += [issue_in(k) for k in range(1, PF)]

        for k in range(K):
            in_t = in_tiles[k]
            in3 = in_t[:].rearrange("p (c x) -> p c x", c=C)

            # diff per plane: shift by one frame (26 elems = 52B) -> 2x
            d_t = pmid.tile([P, C * FO], f16)
            d3 = d_t[:].rearrange("p (c x) -> p c x", c=C)
            for c in range(C):
                nc.vector.tensor_sub(
                    d3[:, c], in3[:, c, NP:], in3[:, c, 0:FO]
                )
                # square in place on the scalar engine
                nc.scalar.activation(d3[:, c], d3[:, c], Af.Square)

            # dist2 = sum over the 3 coord planes: identity matmuls
            # accumulating into PSUM on the otherwise-idle tensor engine
            ps_t = ppsum.tile([P, FO], f32)
            for w0 in range(0, FO, PSUM_W):
                w1 = min(w0 + PSUM_W, FO)
                for c in range(C):
                    nc.tensor.matmul(
                        ps_t[:, w0:w1],
                        id_t[:],
                        d3[:, c, w0:w1],
                        start=(c == 0),
                        stop=(c == C - 1),
                    )

            # dist = sqrt(dist2), draining PSUM (fp32) -> SBUF fp16
            s_t = psm.tile([P, FO], f16)
            nc.scalar.activation(s_t[:], ps_t[:], Af.Sqrt)

            if k + PF < K:
                in_tiles.append(issue_in(k + PF))

            # out_c = x_c + dist; DMA each plane out as soon as it's ready
            o_t = pout.tile([P, C * FO], f16)
            o3 = o_t[:].rearrange("p (c x) -> p c x", c=C)
            for c in range(C):
                nc.vector.tensor_add(o3[:, c], in3[:, c, 0:FO], s_t[:])
                dst = bass.AP(
                    yout,
                    c * OUT_PLANE + k * F * NP,
                    [[C * OUT_PLANE, P], [1, FO]],
                )
                nc.gpsimd.dma_start(dst, o3[:, c])

    nc.compile()
    return nc


def kernel(x: np.ndarray, **_unused) -> np.ndarray:
    x = np.asarray(x)
    assert x.shape == (B, S, N, C), x.shape

    if "nc" not in _cache:
        _cache["nc"] = _build()
    nc = _cache["nc"]

    # Host-side repack: fp16, per (batch, half) partition a c-planar
    # [3, SH+1, 26] block; frame SH is the next real frame (half 0) or a
    # copy of the last frame (half 1) so the device-side distance at the
    # true sequence end is exactly 0.
    xh = np.ascontiguousarray(x).astype(np.float16)          # [B,S,25,3]
    ext = np.concatenate([xh, xh[:, -1:]], axis=1)           # [B,S+1,25,3]
    h0 = ext[:, 0:SH + 1]                                    # [B,513,25,3]
    h1 = ext[:, SH:S + 1]                                    # [B,513,25,3]
    hv = np.stack([h0, h1], axis=1)                          # [B,2,513,25,3]
    pl = np.transpose(hv, (0, 1, 4, 2, 3))                   # [B,2,3,513,25]
    buf = np.zeros((B, H, C, SH + 1, NP), np.float16)
    buf[..., :N] = pl

    eye = np.eye(P, dtype=np.float16).reshape(P * P)
    in_maps = [
        {
            "xin": buf[ci * BC:(ci + 1) * BC].reshape(IN_FLAT),
            "ident": eye,
        }
        for ci in range(NCORES)
    ]

    res = run_bass_kernel_spmd(nc, in_maps, core_ids=list(range(NCORES)))
    _cache["last_results"] = res

    out = np.empty((B, S, N, C), dtype=np.float32)
    for ci in range(NCORES):
        y = np.asarray(res.results[ci]["yout"]).reshape(BC, H, C, SH, NP)
        y = y[..., :N]                                       # strip node pad
        y = np.transpose(y, (0, 1, 3, 4, 2))                 # [BC,2,SH,25,3]
        out[ci * BC:(ci + 1) * BC] = y.reshape(BC, S, N, C).astype(np.float32)
    return out
